# revision 38
# baseline (speedup 1.0000x reference)
"""AttnDecoderRNN step on 8 TRN2 NeuronCores (Bass/Tile).

Sharding (per sharding hint): vocab-parallel out projection (embedding row
handled as a host-side gather/shard selection), hidden-sharded comb/GRU
matmuls, replicated attention. Collectives: AllGather of x (post-comb relu),
AllGather of h', AllGather of per-core log-softmax stats (max, sumexp).

Engine assignment (avoids the gpsimd/SWDGE backlog that delayed collective
triggers): gpsimd carries ONLY collective bounces + triggers + gathered
loads; small inputs ride the scalar HWDGE ring; big weights go first on the
sync ring ahead of the streamed out_W tiles.

Shapes: NHID=1024, NOUT=50257, MAX_LEN=24, batch=1.
Per-core vocab shard: VS=6400 (8*6400=51200 >= 50257; padding gets bias -1e4).
"""
import os
import sys
import types
import contextlib
import ctypes

import numpy as np

# ---------------------------------------------------------------------------
# antenv.axon_hooks shim: the container's antenv stub lacks this module, but
# concourse.bass_utils imports it when tracing is requested (BASS_TRACE=1).
# Provide it, with the ctypes NTFF profile hook libaxon exposes.
# ---------------------------------------------------------------------------
_HOOK = [None]


def _install_axon_hook_shim():
    if "antenv.axon_hooks" not in sys.modules:
        mod = types.ModuleType("antenv.axon_hooks")

        def set_axon_ntff_profile_hook(h):
            _HOOK[0] = h

        def get_axon_ntff_profile_hook():
            return _HOOK[0]

        mod.set_axon_ntff_profile_hook = set_axon_ntff_profile_hook
        mod.get_axon_ntff_profile_hook = get_axon_ntff_profile_hook
        sys.modules["antenv.axon_hooks"] = mod
        try:
            import antenv

            antenv.axon_hooks = mod
        except ImportError:
            pass
    if _HOOK[0] is None:
        so_path = "/opt/axon/libaxon_pjrt.so"
        try:
            lib = ctypes.CDLL(so_path)
        except OSError:
            return
        if not hasattr(lib, "axon_start_nrt_profile"):
            return
        lib.axon_start_nrt_profile.argtypes = [
            ctypes.POINTER(ctypes.c_int64),
            ctypes.c_size_t,
        ]
        lib.axon_start_nrt_profile.restype = ctypes.c_int64
        lib.axon_stop_nrt_profile.argtypes = [ctypes.c_char_p]
        lib.axon_stop_nrt_profile.restype = ctypes.c_int64

        @contextlib.contextmanager
        def _hook(output_dir, device_ids):
            import jax

            jax.devices()
            if device_ids:
                ids = (ctypes.c_int64 * len(device_ids))(*device_ids)
                rc = lib.axon_start_nrt_profile(ids, len(device_ids))
            else:
                rc = lib.axon_start_nrt_profile(None, 0)
            if rc != 0:
                raise RuntimeError(f"axon_start_nrt_profile rc={rc}")
            try:
                yield
            finally:
                n = lib.axon_stop_nrt_profile(str(output_dir).encode())
                print(f"profile: {n} file(s) -> {output_dir}", file=sys.stderr)

        sys.modules["antenv.axon_hooks"].set_axon_ntff_profile_hook(_hook)


_install_axon_hook_shim()

NCORES = 8
NHID = 1024
NOUT = 50257
MAX_LEN = 24
HC = NHID // 128          # 8 hidden chunks of 128
VS = 6400                 # vocab rows per core (padded)
TN = 400                  # out-projection free-dim tile (PSUM bank limit: 512 f32)
NT = VS // TN             # 16 logical tiles per core
WG = 2                    # logical tiles per W DMA chunk
PAD_BIAS = -1.0e4         # bias on padded vocab rows: exp() underflows to 0
WOUT_BF16 = os.environ.get("WOUT_DTYPE", "bf16") == "bf16"

_CACHE = {}


def _build():
    import concourse.bass as bass
    import concourse.tile as tile
    from concourse import bacc, mybir, masks
    from contextlib import ExitStack

    f32 = mybir.dt.float32
    wdt = mybir.dt.bfloat16 if WOUT_BF16 else f32

    nc = bacc.Bacc(
        "TRN2",
        target_bir_lowering=False,
        debug=False,
        enable_asserts=True,
        num_devices=NCORES,
    )

    # ---- I/O ----
    # mega-pack: pk [0:65] | attnw [65:449] | combw [449:2497] | pr row0 [2497:2649]
    pack_in = nc.dram_tensor("pack_in", [128, 2649], f32, kind="ExternalInput")
    enc_in = nc.dram_tensor("enc_in", [MAX_LEN, NHID], f32, kind="ExternalInput")
    wih_in = nc.dram_tensor("wih_in", [128, 3 * NHID], mybir.dt.bfloat16, kind="ExternalInput")
    whh_in = nc.dram_tensor("whh_in", [128, 3 * NHID], mybir.dt.bfloat16, kind="ExternalInput")
    wout_in = nc.dram_tensor(
        "wout_in", [NT // WG, 128, WG * HC * TN], wdt, kind="ExternalInput"
    )
    bout_in = nc.dram_tensor("bout_in", [16, TN], f32, kind="ExternalInput")

    logp_out = nc.dram_tensor("logp_out", [16, TN], f32, kind="ExternalOutput")
    h_out = nc.dram_tensor("h_out", [128, HC], f32, kind="ExternalOutput")
    attn_out = nc.dram_tensor("attn_out", [1, MAX_LEN], f32, kind="ExternalOutput")

    RG = [list(range(NCORES))]

    with tile.TileContext(nc) as tc:
        with ExitStack() as ctx:
            wpool = ctx.enter_context(tc.tile_pool(name="wpool", bufs=8))
            cpool = ctx.enter_context(tc.tile_pool(name="cpool", bufs=1))
            spool = ctx.enter_context(tc.tile_pool(name="spool", bufs=1))
            pp = ctx.enter_context(tc.tile_pool(name="pp", bufs=2, space="PSUM"))
            dram = ctx.enter_context(tc.tile_pool(name="dram", bufs=1, space="DRAM"))

            # ---- constants / inputs to SBUF ----
            # chain-critical inputs FIRST on the sync ring (ahead of the W
            # stream); non-critical small ones on the scalar ring
            ident = cpool.tile([48, 48], f32)
            masks.make_identity(nc, ident[:])
            ones_row = cpool.tile([1, 16], f32)    # [1,P] lhsT for broadcasts
            nc.gpsimd.memset(ones_row[:], 1.0)
            ones_col = cpool.tile([16, 1], f32)    # [P,1] rhs for partition sums
            nc.gpsimd.memset(ones_col[:], 1.0)

            psW = pp.tile([48, 48], f32, tag="psW", bufs=1)
            for _ in range(45):
                nc.tensor.matmul(psW[:], ident[:], ident[:], start=True, stop=True)

            pack_sb = cpool.tile([128, 2649], f32)
            nc.sync.dma_start(pack_sb[:], pack_in[:])
            emb_sb = pack_sb[:, 0:HC]
            h0c_sb = pack_sb[:, HC : 2 * HC]
            h0own_sb = pack_sb[:, 2 * HC : 2 * HC + 1]
            bih_sb = pack_sb[:, 17:41]
            bhh_sb = pack_sb[:, 41:65]
            attnw_sb = pack_sb[:, 65:449]
            combw_sb = pack_sb[:, 449:2497]
            attnb_sb = pack_sb[0:1, 2497 : 2497 + MAX_LEN]
            combb_sb = pack_sb[0:1, 2497 + MAX_LEN : 2649]
            enc_sb = cpool.tile([MAX_LEN, NHID], f32)
            nc.sync.dma_start(enc_sb[:], enc_in[:])
            wih_sb = cpool.tile([128, 3 * NHID], mybir.dt.bfloat16)
            nc.sync.dma_start(wih_sb[:], wih_in[:])
            whh_sb = cpool.tile([128, 3 * NHID], mybir.dt.bfloat16)
            nc.sync.dma_start(whh_sb[:], whh_in[:])
            bout_sb = cpool.tile([16, TN], f32)
            nc.sync.dma_start(bout_sb[:], bout_in[:])



            # ---- attention (replicated) ----
            psA = pp.tile([128, MAX_LEN], f32, tag="psA", bufs=2)
            alog_ps = psA[0:1, 0:MAX_LEN]
            for c in range(HC):
                nc.tensor.matmul(
                    alog_ps,
                    emb_sb[:, c : c + 1],
                    attnw_sb[:, c * MAX_LEN : (c + 1) * MAX_LEN],
                    start=(c == 0),
                    stop=False,
                )
            for c in range(HC):
                nc.tensor.matmul(
                    alog_ps,
                    h0c_sb[:, c : c + 1],
                    attnw_sb[:, (HC + c) * MAX_LEN : (HC + c + 1) * MAX_LEN],
                    start=False,
                    stop=(c == HC - 1),
                )
            alog_sb = spool.tile([1, MAX_LEN], f32)
            nc.vector.tensor_add(alog_sb[:], alog_ps, attnb_sb[:])
            amax = spool.tile([1, 1], f32)
            nc.vector.reduce_max(amax[:], alog_sb[:], axis=mybir.AxisListType.X)
            namax = spool.tile([1, 1], f32)
            nc.vector.tensor_scalar_mul(namax[:], amax[:], -1.0)
            probs = spool.tile([1, MAX_LEN], f32)
            sume = spool.tile([1, 1], f32)
            nc.scalar.activation(
                probs[:], alog_sb[:], mybir.ActivationFunctionType.Exp,
                bias=namax[0:1, 0:1], accum_out=sume[:],
            )
            rinv = spool.tile([1, 1], f32)
            nc.vector.reciprocal(rinv[:], sume[:])
            attnp_sb = spool.tile([1, MAX_LEN], f32)
            nc.scalar.mul(attnp_sb[:], probs[:], rinv[0:1, 0:1])
            nc.scalar.dma_start(attn_out[:], attnp_sb[:])

            # transpose attn probs -> [24, 1]
            psS = pp.tile([128, 16], f32, tag="psS", bufs=1)
            nc.tensor.transpose(psS[0:MAX_LEN, 0:1], attnp_sb[:], ident[0:1, 0:1])
            awt_sb = spool.tile([MAX_LEN, 1], f32)
            nc.vector.tensor_copy(awt_sb[:], psS[0:MAX_LEN, 0:1])

            # attn_applied chunks: [128, HC]
            psA2 = pp.tile([128, MAX_LEN], f32, tag="psA", bufs=2)
            for c in range(HC):
                nc.tensor.matmul(
                    psA2[:, c : c + 1],
                    enc_sb[0:MAX_LEN, c * 128 : (c + 1) * 128],
                    awt_sb[:],
                    start=True,
                    stop=True,
                )
            aap_sb = spool.tile([128, HC], f32)
            nc.vector.tensor_copy(aap_sb[:], psA2[:, 0:HC])

            # ---- comb (sharded, thin-stationary): x row = relu(cat @ W_sh.T + b)
            # lhsT = cat chunk [128,1] (trivial weight load), rhs = W tile.
            psC = pp.tile([1, 512], f32, tag="psG", bufs=2)
            psC_ap = psC[0:1, 0:128]
            for c in range(HC):
                nc.tensor.matmul(
                    psC_ap, emb_sb[:, c : c + 1],
                    combw_sb[:, c * 128 : (c + 1) * 128],
                    start=(c == 0), stop=False)
            for c in range(HC):
                nc.tensor.matmul(
                    psC_ap, aap_sb[:, c : c + 1],
                    combw_sb[:, (HC + c) * 128 : (HC + c + 1) * 128],
                    start=False, stop=False)
            # bias via K=1 ones matmul, then relu
            nc.tensor.matmul(psC_ap, ones_row[0:1, 0:1], combb_sb[:],
                             start=False, stop=True)
            xrow_sb = spool.tile([1, 128], f32)
            nc.scalar.activation(
                xrow_sb[:], psC_ap, mybir.ActivationFunctionType.Relu)
            # transpose x row -> [128, 1] for use as stationary operand
            psX = pp.tile([128, 48], f32, tag="psA", bufs=2)
            nc.tensor.transpose(psX[:, 0:1], xrow_sb[:], ident[0:1, 0:1])
            xsh_sb = spool.tile([128, 1], mybir.dt.bfloat16)
            nc.vector.tensor_copy(xsh_sb[:], psX[:, 0:1])
            h0own_bf = spool.tile([128, 1], mybir.dt.bfloat16)
            nc.vector.tensor_copy(h0own_bf[:], h0own_sb[:])

            # ---- GRU, contraction-sharded with weights as the moving
            # operand: partial gi = x_chunk.T @ W_ih[:, j].T -> [1, 3072],
            # same for gh; AllReduce(add) of [1, 6144]; transpose-load to
            # [128, 48] chunk layout; elementwise GRU gives FULL h'
            # replicated on every core (no h all-gather).
            # 12 pieces (gi n=0..5, gh n=0..5) in 3 waves of 4 concurrent
            # column-strips; each wave drains with one wide DVE copy and four
            # direct SBUF->DRAM writes into the AllReduce bounce
            ar_in = dram.tile([1, 2 * 3 * NHID], f32)
            ar_out = dram.tile([1, 2 * 3 * NHID], f32, addr_space="Shared")
            for w in range(3):
                psG4 = pp.tile([128, 512], f32, tag="psG", bufs=2)
                for q in range(4):
                    p = 4 * w + q
                    lhs = xsh_sb if p < 6 else h0own_bf
                    n = p % 6
                    wmat = wih_sb if p < 6 else whh_sb
                    nc.tensor.matmul(
                        psG4[32 * q : 32 * q + 1, :],
                        lhs[:], wmat[:, n * 512 : (n + 1) * 512],
                        start=True, stop=True, tile_position=(0, 32 * q))
                gw = spool.tile([128, 512], f32, tag="gw", bufs=2)
                nc.vector.tensor_copy(gw[:], psG4[:])
                for q in range(4):
                    p = 4 * w + q
                    nc.scalar.dma_start(
                        ar_in[0:1, p * 512 : (p + 1) * 512],
                        gw[32 * q : 32 * q + 1, :])
            nc.gpsimd.collective_compute(
                "AllReduce", mybir.AluOpType.add, replica_groups=RG,
                ins=[ar_in[:].opt()], outs=[ar_out[:].opt()],
            )
            # load as [48, 128] (contiguous rows) and transpose to [128, 48]
            g48_sb = spool.tile([48, 128], f32)
            nc.gpsimd.dma_start(g48_sb[:], ar_out[:].rearrange("o (a b) -> (o a) b", b=128))
            psG48 = pp.tile([128, 48], f32, tag="psA", bufs=2)
            nc.tensor.transpose(psG48[:], g48_sb[:], ident[0:48, 0:48])
            gf_sb = spool.tile([128, 48], f32)
            nc.vector.tensor_copy(gf_sb[:], psG48[:])
            # cols: 0:8 gi_r, 8:16 gi_z, 16:24 gi_n, 24:32 gh_r, 32:40 gh_z, 40:48 gh_n

            # elementwise GRU on [128, HC] chunk-layout tensors
            brz_sb = spool.tile([128, 2 * HC], f32)
            nc.vector.tensor_add(brz_sb[:], bih_sb[:, 0 : 2 * HC], bhh_sb[:, 0 : 2 * HC])
            rz0_sb = spool.tile([128, 2 * HC], f32)
            nc.vector.tensor_add(rz0_sb[:], gf_sb[:, 0 : 2 * HC], gf_sb[:, 3 * HC : 5 * HC])
            rzin_sb = spool.tile([128, 2 * HC], f32)
            nc.vector.tensor_add(rzin_sb[:], rz0_sb[:], brz_sb[:])
            rz_sb = spool.tile([128, 2 * HC], f32)
            nc.scalar.activation(
                rz_sb[:], rzin_sb[:], mybir.ActivationFunctionType.Sigmoid)
            hnb_sb = spool.tile([128, HC], f32)
            nc.vector.tensor_add(
                hnb_sb[:], gf_sb[:, 5 * HC : 6 * HC], bhh_sb[:, 2 * HC : 3 * HC])
            rhn_sb = spool.tile([128, HC], f32)
            nc.vector.tensor_mul(rhn_sb[:], rz_sb[:, 0:HC], hnb_sb[:])
            t1_sb = spool.tile([128, HC], f32)
            nc.vector.tensor_add(t1_sb[:], gf_sb[:, 2 * HC : 3 * HC], rhn_sb[:])
            t2_sb = spool.tile([128, HC], f32)
            nc.vector.tensor_add(t2_sb[:], t1_sb[:], bih_sb[:, 2 * HC : 3 * HC])
            n_sb = spool.tile([128, HC], f32)
            nc.scalar.activation(
                n_sb[:], t2_sb[:], mybir.ActivationFunctionType.Tanh)
            d_sb = spool.tile([128, HC], f32)
            nc.vector.tensor_sub(d_sb[:], h0c_sb[:], n_sb[:])
            zd_sb = spool.tile([128, HC], f32)
            nc.vector.tensor_mul(zd_sb[:], rz_sb[:, HC : 2 * HC], d_sb[:])
            hn_sb = spool.tile([128, HC], f32)
            nc.vector.tensor_add(hn_sb[:], n_sb[:], zd_sb[:])
            nc.scalar.dma_start(h_out[:], hn_sb[:])
            # preload Exp/Ln activation tables off the critical tail
            tbl_scr = spool.tile([1, 1], f32)
            nc.scalar.activation(tbl_scr[:], hn_sb[0:1, 0:1],
                                 mybir.ActivationFunctionType.Exp)
            tbl_scr2 = spool.tile([1, 1], f32)
            nc.scalar.activation(tbl_scr2[:], ones_row[0:1, 0:1],
                                 mybir.ActivationFunctionType.Ln)
            h_mm = cpool.tile([128, HC], wdt)
            nc.vector.tensor_copy(h_mm[:], hn_sb[:])

            # ---- out projection (streamed, vocab shard VS=6400, 16 tiles)
            # bias folded into the matmul accumulation via a K=1 ones matmul;
            # per-tile online softmax stats on partition 0 overlap the stream.
            logits_sb = cpool.tile([16, TN], f32)
            for quad in range(NT // 4):
                w_a = wpool.tile([128, WG * HC * TN], wdt, tag="wtile")
                nc.sync.dma_start(w_a[:], wout_in[2 * quad])
                w_b = wpool.tile([128, WG * HC * TN], wdt, tag="wtile")
                nc.sync.dma_start(w_b[:], wout_in[2 * quad + 1])
                # 4-way PE column-tiling: 4 tiles run in disjoint 32-column
                # strips of the array concurrently
                psT4 = pp.tile([128, TN], f32, tag="psT", bufs=2)
                for k in range(HC):
                    for q in range(4):
                        w_t = w_a if q < 2 else w_b
                        nc.tensor.matmul(
                            psT4[32 * q : 32 * q + 1, :],
                            h_mm[:, k : k + 1],
                            w_t[:, ((q % 2) * HC + k) * TN : ((q % 2) * HC + k + 1) * TN],
                            start=(k == 0),
                            stop=(k == HC - 1),
                            tile_position=(0, 32 * q),
                        )
                # one full-width DVE copy drains all 4 rows (lanes in parallel)
                lwide = spool.tile([128, TN], f32, tag="lwide", bufs=2)
                nc.vector.tensor_copy(lwide[:], psT4[:])
                for q in range(4):
                    t = quad * 4 + q
                    # sync ring is idle once the W chunks are issued
                    nc.sync.dma_start(
                        logits_sb[t : t + 1, :], lwide[32 * q : 32 * q + 1, :])

            # ---- combine the 16 per-tile stats (partition 0) ----
            logits2_sb = cpool.tile([16, TN], f32)
            nc.vector.tensor_add(logits2_sb[:], logits_sb[:], bout_sb[:])
            nm16 = spool.tile([16, 1], f32)   # -max per row
            nc.vector.reduce_max(
                nm16[:], logits2_sb[:], axis=mybir.AxisListType.X, negate=True)
            e16 = spool.tile([16, TN], f32, tag="row16", bufs=1)
            zrow16 = spool.tile([16, 1], f32)
            nc.scalar.activation(
                e16[:], logits2_sb[:], mybir.ActivationFunctionType.Exp,
                bias=nm16[:, 0:1], accum_out=zrow16[:])
            psMT = pp.tile([128, 16], f32, tag="psS", bufs=1)
            nc.tensor.transpose(psMT[0:1, 0:16], nm16[:], ident[0:16, 0:16])
            mt_sb = spool.tile([1, 16], f32)
            nc.vector.tensor_copy(mt_sb[:], psMT[0:1, 0:16])
            nmloc = spool.tile([1, 1], f32)   # = -M_loc
            nc.vector.tensor_reduce(
                nmloc[:], mt_sb[:], op=mybir.AluOpType.min,
                axis=mybir.AxisListType.X)
            psNB = pp.tile([16, 1], f32, tag="psS", bufs=1)
            nc.tensor.matmul(psNB[:], ones_row[0:1, 0:16], nmloc[:],
                             start=True, stop=True)
            nmlb = spool.tile([16, 1], f32)
            nc.vector.tensor_copy(nmlb[:], psNB[:])
            m16p = spool.tile([16, 1], f32)
            nc.vector.tensor_scalar_mul(m16p[:], nm16[:], -1.0)
            etp = spool.tile([16, 1], f32)    # exp(m_t - M_loc)
            nc.scalar.activation(
                etp[:], m16p[:], mybir.ActivationFunctionType.Exp,
                bias=nmlb[:, 0:1])
            zs16 = spool.tile([16, 1], f32)
            nc.vector.tensor_mul(zs16[:], etp[:], zrow16[:])
            psZl = pp.tile([1, 1], f32, tag="psS", bufs=1)
            nc.tensor.matmul(psZl[:], zs16[:], ones_col[0:16, 0:1],
                             start=True, stop=True)
            stats_sb = spool.tile([1, 2], f32)
            nc.vector.tensor_scalar_mul(stats_sb[0:1, 0:1], nmloc[:], -1.0)
            nc.vector.tensor_copy(stats_sb[0:1, 1:2], psZl[:])

            # ---- AllGather stats ----
            st_in = dram.tile([1, 2], f32)
            st_out = dram.tile([NCORES, 2], f32, addr_space="Shared")
            nc.gpsimd.dma_start(st_in[:], stats_sb[:])
            nc.gpsimd.collective_compute(
                "AllGather", mybir.AluOpType.bypass, replica_groups=RG,
                ins=[st_in[:].opt()], outs=[st_out[:].opt()],
            )
            s16_sb = spool.tile([1, 2 * NCORES], f32)
            nc.gpsimd.dma_start(
                s16_sb[:], st_out[:].rearrange("(o a) b -> o (a b)", o=1))

            # global max M, then C = M + ln(sum_c Z_c exp(m_c - M))
            s16v = s16_sb[:].rearrange("p (a b) -> p a b", b=2)
            ngM = spool.tile([1, 1], f32)
            nc.vector.reduce_max(ngM[:], s16v[:, :, 0:1],
                                 axis=mybir.AxisListType.XY, negate=True)
            e8 = spool.tile([1, NCORES], f32)
            nc.scalar.activation(
                e8[:], s16v[:, :, 0:1], mybir.ActivationFunctionType.Exp,
                bias=ngM[0:1, 0:1])
            s8p = spool.tile([1, NCORES], f32)
            nc.vector.tensor_mul(s8p[:], e8[:], s16v[:, :, 1:2])
            Zg = spool.tile([1, 1], f32)
            nc.vector.reduce_sum(Zg[:], s8p[:], axis=mybir.AxisListType.X)
            lnZ = spool.tile([1, 1], f32)
            nc.scalar.activation(lnZ[:], Zg[:], mybir.ActivationFunctionType.Ln)
            nC = spool.tile([1, 1], f32)
            nc.vector.tensor_sub(nC[:], ngM[:], lnZ[:])
            psB3 = pp.tile([16, 1], f32, tag="psS", bufs=1)
            nc.tensor.matmul(psB3[:], ones_row[0:1, 0:16], nC[:], start=True, stop=True)
            nC16_sb = spool.tile([16, 1], f32)
            nc.vector.tensor_copy(nC16_sb[:], psB3[:])
            logp_sb = spool.tile([16, TN], f32, tag="row16", bufs=1)
            nc.scalar.activation(
                logp_sb[:], logits2_sb[:], mybir.ActivationFunctionType.Identity,
                bias=nC16_sb[:, 0:1])
            nc.sync.dma_start(logp_out[:], logp_sb[:])

    nc.compile()
    return nc


def _prep_inputs(inp, hidden, encoder_outputs, emb_W, attn_W, attn_b,
                 comb_W, comb_b, W_ih, W_hh, b_ih, b_hh, out_W, out_b):
    """Shard/layout the full inputs into 8 per-core input maps."""
    f = np.float32
    idx = int(np.asarray(inp).ravel()[0])
    emb_row = np.asarray(emb_W[idx], dtype=f)                 # [1024]
    h0 = np.asarray(hidden, dtype=f).ravel()                  # [1024]
    enc = np.ascontiguousarray(np.asarray(encoder_outputs, dtype=f))  # [24,1024]

    def chunked_vec(v):
        # [1024] -> [128, 8] with [p, c] = v[c*128+p]
        return np.ascontiguousarray(v.reshape(HC, 128).T)

    emb_c = chunked_vec(emb_row)
    h0_c = chunked_vec(h0)

    # attn_W [24, 2048] -> [128, 16*24]
    aT = np.asarray(attn_W, dtype=f).T.reshape(16, 128, MAX_LEN)
    attnw = np.ascontiguousarray(aT.transpose(1, 0, 2).reshape(128, 16 * MAX_LEN))
    attnb = np.ascontiguousarray(np.asarray(attn_b, dtype=f).reshape(1, MAX_LEN))

    comb_W = np.asarray(comb_W, dtype=f)
    comb_b_a = np.asarray(comb_b, dtype=f)
    W_ih_a = np.asarray(W_ih, dtype=f)
    W_hh_a = np.asarray(W_hh, dtype=f)
    b_ih_a = np.asarray(b_ih, dtype=f)
    b_hh_a = np.asarray(b_hh, dtype=f)
    out_W_a = np.asarray(out_W, dtype=f)
    out_b_a = np.asarray(out_b, dtype=f)

    if WOUT_BF16:
        import ml_dtypes

        wout_dt = ml_dtypes.bfloat16
    else:
        wout_dt = f

    in_maps = []
    for j in range(NCORES):
        sl = slice(j * 128, (j + 1) * 128)
        # comb shard [128, 2048] -> [128(p), 16*128]
        cw = comb_W[sl]                                    # [128, 2048]
        cwT = cw.T.reshape(16, 128, 128)                   # [c, p, m]
        combw = np.ascontiguousarray(cwT.transpose(1, 0, 2).reshape(128, 16 * 128))
        combb = np.ascontiguousarray(comb_b_a[sl].reshape(1, 128))

        def gate_pack(W):
            # moving-operand layout: rhs[p, q] = W[q, j*128+p]
            import ml_dtypes
            return np.ascontiguousarray(
                W[:, j * 128 : (j + 1) * 128].T.astype(ml_dtypes.bfloat16))

        wih = gate_pack(W_ih_a)
        whh = gate_pack(W_hh_a)

        def bias_pack(b):
            # [128, 3*HC]: col g*HC+c holds b[g*1024 + c*128 + p]
            return np.ascontiguousarray(
                b.reshape(3, HC, 128).transpose(2, 0, 1).reshape(128, 3 * HC))

        bih = bias_pack(b_ih_a)
        bhh = bias_pack(b_hh_a)

        # out_W vocab shard [VS, 1024] (zero-padded), bias shard with PAD_BIAS
        lo, hi = j * VS, min((j + 1) * VS, NOUT)
        nreal = max(0, hi - lo)
        wsh = np.zeros((VS, NHID), dtype=f)
        bsh = np.full((VS,), PAD_BIAS, dtype=f)
        if nreal > 0:
            wsh[:nreal] = out_W_a[lo:hi]
            bsh[:nreal] = out_b_a[lo:hi]
        WT = wsh.T                                         # [1024, 6400]
        # [NT, 128, HC*TN]: [t, p, k*TN+n] = WT[k*128+p, t*TN+n], grouped by WG
        warr = (
            WT.reshape(HC, 128, NT, TN).transpose(2, 1, 0, 3)
            .reshape(NT // WG, WG, 128, HC * TN).transpose(0, 2, 1, 3)
            .reshape(NT // WG, 128, WG * HC * TN)
        )
        warr = np.ascontiguousarray(warr.astype(wout_dt))
        barr = np.ascontiguousarray(bsh.reshape(16, TN))

        pack = np.zeros((128, 2649), dtype=f)
        pack[:, 0:HC] = emb_c
        pack[:, HC : 2 * HC] = h0_c
        pack[:, 2 * HC : 2 * HC + 1] = h0[sl].reshape(128, 1)
        pack[:, 17:41] = bih
        pack[:, 41:65] = bhh
        pack[:, 65:449] = attnw
        pack[:, 449:2497] = combw
        pack[0, 2497 : 2497 + MAX_LEN] = attnb.ravel()
        pack[0, 2497 + MAX_LEN : 2649] = combb.ravel()
        in_maps.append({
            "pack_in": np.ascontiguousarray(pack),
            "enc_in": enc,
            "wih_in": wih, "whh_in": whh,
            "wout_in": warr, "bout_in": barr,
        })
    return in_maps


def run(trace=False, **inputs):
    from concourse.bass_utils import run_bass_kernel_spmd

    if "nc" not in _CACHE:
        _CACHE["nc"] = _build()
    nc = _CACHE["nc"]

    inputs.pop("encoder_output", None)  # unused by the reference computation
    in_maps = _prep_inputs(**inputs)
    res = run_bass_kernel_spmd(
        nc, in_maps, core_ids=list(range(NCORES)), trace=trace
    )

    logp = np.concatenate(
        [res.results[j]["logp_out"].reshape(-1) for j in range(NCORES)]
    )[:NOUT].reshape(1, NOUT).astype(np.float32)
    # h_out is [128, HC] chunk layout, full h' replicated on every core
    h = res.results[0]["h_out"].T.reshape(1, 1, NHID).astype(np.float32)
    attn = res.results[0]["attn_out"].reshape(1, MAX_LEN).astype(np.float32)
    return (logp, h, attn), res


def kernel(**inputs):
    out, _ = run(trace=bool(os.environ.get("KERNEL_TRACE")), **inputs)
    return out


# revision 39
# speedup vs baseline: 1.3887x; 1.3887x over previous
"""AttnDecoderRNN step on 8 TRN2 NeuronCores (Bass/Tile).

Sharding (per sharding hint): vocab-parallel out projection (embedding row
handled as a host-side gather/shard selection), hidden-sharded comb/GRU
matmuls, replicated attention. Collectives: AllGather of x (post-comb relu),
AllGather of h', AllGather of per-core log-softmax stats (max, sumexp).

Engine assignment (avoids the gpsimd/SWDGE backlog that delayed collective
triggers): gpsimd carries ONLY collective bounces + triggers + gathered
loads; small inputs ride the scalar HWDGE ring; big weights go first on the
sync ring ahead of the streamed out_W tiles.

Shapes: NHID=1024, NOUT=50257, MAX_LEN=24, batch=1.
Per-core vocab shard: VS=6400 (8*6400=51200 >= 50257; padding gets bias -1e4).
"""
import os
import sys
import types
import contextlib
import ctypes

import numpy as np

# ---------------------------------------------------------------------------
# antenv.axon_hooks shim: the container's antenv stub lacks this module, but
# concourse.bass_utils imports it when tracing is requested (BASS_TRACE=1).
# Provide it, with the ctypes NTFF profile hook libaxon exposes.
# ---------------------------------------------------------------------------
_HOOK = [None]


def _install_axon_hook_shim():
    if "antenv.axon_hooks" not in sys.modules:
        mod = types.ModuleType("antenv.axon_hooks")

        def set_axon_ntff_profile_hook(h):
            _HOOK[0] = h

        def get_axon_ntff_profile_hook():
            return _HOOK[0]

        mod.set_axon_ntff_profile_hook = set_axon_ntff_profile_hook
        mod.get_axon_ntff_profile_hook = get_axon_ntff_profile_hook
        sys.modules["antenv.axon_hooks"] = mod
        try:
            import antenv

            antenv.axon_hooks = mod
        except ImportError:
            pass
    if _HOOK[0] is None:
        so_path = "/opt/axon/libaxon_pjrt.so"
        try:
            lib = ctypes.CDLL(so_path)
        except OSError:
            return
        if not hasattr(lib, "axon_start_nrt_profile"):
            return
        lib.axon_start_nrt_profile.argtypes = [
            ctypes.POINTER(ctypes.c_int64),
            ctypes.c_size_t,
        ]
        lib.axon_start_nrt_profile.restype = ctypes.c_int64
        lib.axon_stop_nrt_profile.argtypes = [ctypes.c_char_p]
        lib.axon_stop_nrt_profile.restype = ctypes.c_int64

        @contextlib.contextmanager
        def _hook(output_dir, device_ids):
            import jax

            jax.devices()
            if device_ids:
                ids = (ctypes.c_int64 * len(device_ids))(*device_ids)
                rc = lib.axon_start_nrt_profile(ids, len(device_ids))
            else:
                rc = lib.axon_start_nrt_profile(None, 0)
            if rc != 0:
                raise RuntimeError(f"axon_start_nrt_profile rc={rc}")
            try:
                yield
            finally:
                n = lib.axon_stop_nrt_profile(str(output_dir).encode())
                print(f"profile: {n} file(s) -> {output_dir}", file=sys.stderr)

        sys.modules["antenv.axon_hooks"].set_axon_ntff_profile_hook(_hook)


_install_axon_hook_shim()

NCORES = 8
NHID = 1024
NOUT = 50257
MAX_LEN = 24
HC = NHID // 128          # 8 hidden chunks of 128
VS = 6400                 # vocab rows per core (padded)
TN = 400                  # out-projection free-dim tile (PSUM bank limit: 512 f32)
NT = VS // TN             # 16 logical tiles per core
WG = 2                    # logical tiles per W DMA chunk
PAD_BIAS = -1.0e4         # bias on padded vocab rows: exp() underflows to 0
WOUT_BF16 = os.environ.get("WOUT_DTYPE", "bf16") == "bf16"

_CACHE = {}


def _build():
    import concourse.bass as bass
    import concourse.tile as tile
    from concourse import bacc, mybir, masks
    from contextlib import ExitStack

    f32 = mybir.dt.float32
    wdt = mybir.dt.bfloat16 if WOUT_BF16 else f32

    nc = bacc.Bacc(
        "TRN2",
        target_bir_lowering=False,
        debug=False,
        enable_asserts=True,
        num_devices=NCORES,
    )

    # ---- I/O ----
    # mega-pack: pk [0:65] | attnw [65:449] | combw [449:2497] | pr row0 [2497:2649]
    pack_in = nc.dram_tensor("pack_in", [128, 2649], f32, kind="ExternalInput")
    enc_in = nc.dram_tensor("enc_in", [MAX_LEN, NHID], f32, kind="ExternalInput")
    wih_in = nc.dram_tensor("wih_in", [128, 3 * NHID], mybir.dt.bfloat16, kind="ExternalInput")
    whh_in = nc.dram_tensor("whh_in", [128, 3 * NHID], mybir.dt.bfloat16, kind="ExternalInput")
    wout_in = nc.dram_tensor(
        "wout_in", [NT // WG, 128, WG * HC * TN], wdt, kind="ExternalInput"
    )
    bout_in = nc.dram_tensor("bout_in", [16, TN], f32, kind="ExternalInput")

    logp_out = nc.dram_tensor("logp_out", [16, TN], f32, kind="ExternalOutput")
    h_out = nc.dram_tensor("h_out", [128, HC], f32, kind="ExternalOutput")
    attn_out = nc.dram_tensor("attn_out", [1, MAX_LEN], f32, kind="ExternalOutput")

    RG = [list(range(NCORES))]

    with tile.TileContext(nc) as tc:
        with ExitStack() as ctx:
            wpool = ctx.enter_context(tc.tile_pool(name="wpool", bufs=8))
            cpool = ctx.enter_context(tc.tile_pool(name="cpool", bufs=1))
            spool = ctx.enter_context(tc.tile_pool(name="spool", bufs=1))
            pp = ctx.enter_context(tc.tile_pool(name="pp", bufs=2, space="PSUM"))
            dram = ctx.enter_context(tc.tile_pool(name="dram", bufs=1, space="DRAM"))

            # ---- constants / inputs to SBUF ----
            # chain-critical inputs FIRST on the sync ring (ahead of the W
            # stream); non-critical small ones on the scalar ring
            ident = cpool.tile([48, 48], f32)
            masks.make_identity(nc, ident[:])
            ones_row = cpool.tile([1, 16], f32)    # [1,P] lhsT for broadcasts
            nc.gpsimd.memset(ones_row[:], 1.0)
            ones_col = cpool.tile([16, 1], f32)    # [P,1] rhs for partition sums
            nc.gpsimd.memset(ones_col[:], 1.0)

            psW = pp.tile([48, 48], f32, tag="psW", bufs=1)
            for _ in range(45):
                nc.tensor.matmul(psW[:], ident[:], ident[:], start=True, stop=True)

            pack_sb = cpool.tile([128, 2649], f32)
            nc.sync.dma_start(pack_sb[:], pack_in[:])
            emb_sb = pack_sb[:, 0:HC]
            h0c_sb = pack_sb[:, HC : 2 * HC]
            h0own_sb = pack_sb[:, 2 * HC : 2 * HC + 1]
            bih_sb = pack_sb[:, 17:41]
            bhh_sb = pack_sb[:, 41:65]
            attnw_sb = pack_sb[:, 65:449]
            combw_sb = pack_sb[:, 449:2497]
            attnb_sb = pack_sb[0:1, 2497 : 2497 + MAX_LEN]
            combb_sb = pack_sb[0:1, 2497 + MAX_LEN : 2649]
            enc_sb = cpool.tile([MAX_LEN, NHID], f32)
            nc.sync.dma_start(enc_sb[:], enc_in[:])
            wih_sb = cpool.tile([128, 3 * NHID], mybir.dt.bfloat16)
            nc.sync.dma_start(wih_sb[:], wih_in[:])
            whh_sb = cpool.tile([128, 3 * NHID], mybir.dt.bfloat16)
            nc.sync.dma_start(whh_sb[:], whh_in[:])
            bout_sb = cpool.tile([16, TN], f32)
            nc.sync.dma_start(bout_sb[:], bout_in[:])



            # ---- attention (replicated) ----
            psA = pp.tile([128, MAX_LEN], f32, tag="psA", bufs=2)
            alog_ps = psA[0:1, 0:MAX_LEN]
            for c in range(HC):
                nc.tensor.matmul(
                    alog_ps,
                    emb_sb[:, c : c + 1],
                    attnw_sb[:, c * MAX_LEN : (c + 1) * MAX_LEN],
                    start=(c == 0),
                    stop=False,
                )
            for c in range(HC):
                nc.tensor.matmul(
                    alog_ps,
                    h0c_sb[:, c : c + 1],
                    attnw_sb[:, (HC + c) * MAX_LEN : (HC + c + 1) * MAX_LEN],
                    start=False,
                    stop=(c == HC - 1),
                )
            alog_sb = spool.tile([1, MAX_LEN], f32)
            nc.vector.tensor_add(alog_sb[:], alog_ps, attnb_sb[:])
            amax = spool.tile([1, 1], f32)
            nc.vector.reduce_max(amax[:], alog_sb[:], axis=mybir.AxisListType.X)
            namax = spool.tile([1, 1], f32)
            nc.vector.tensor_scalar_mul(namax[:], amax[:], -1.0)
            probs = spool.tile([1, MAX_LEN], f32)
            sume = spool.tile([1, 1], f32)
            nc.scalar.activation(
                probs[:], alog_sb[:], mybir.ActivationFunctionType.Exp,
                bias=namax[0:1, 0:1], accum_out=sume[:],
            )
            rinv = spool.tile([1, 1], f32)
            nc.vector.reciprocal(rinv[:], sume[:])
            attnp_sb = spool.tile([1, MAX_LEN], f32)
            nc.scalar.mul(attnp_sb[:], probs[:], rinv[0:1, 0:1])
            nc.scalar.dma_start(attn_out[:], attnp_sb[:])

            # transpose attn probs -> [24, 1]
            psS = pp.tile([128, 16], f32, tag="psS", bufs=1)
            nc.tensor.transpose(psS[0:MAX_LEN, 0:1], attnp_sb[:], ident[0:1, 0:1])
            awt_sb = spool.tile([MAX_LEN, 1], f32)
            nc.vector.tensor_copy(awt_sb[:], psS[0:MAX_LEN, 0:1])

            # attn_applied chunks: [128, HC]
            psA2 = pp.tile([128, MAX_LEN], f32, tag="psA", bufs=2)
            for c in range(HC):
                nc.tensor.matmul(
                    psA2[:, c : c + 1],
                    enc_sb[0:MAX_LEN, c * 128 : (c + 1) * 128],
                    awt_sb[:],
                    start=True,
                    stop=True,
                )
            aap_sb = spool.tile([128, HC], f32)
            nc.vector.tensor_copy(aap_sb[:], psA2[:, 0:HC])

            # ---- comb (sharded, thin-stationary): x row = relu(cat @ W_sh.T + b)
            # lhsT = cat chunk [128,1] (trivial weight load), rhs = W tile.
            psC = pp.tile([1, 512], f32, tag="psG", bufs=2)
            psC_ap = psC[0:1, 0:128]
            for c in range(HC):
                nc.tensor.matmul(
                    psC_ap, emb_sb[:, c : c + 1],
                    combw_sb[:, c * 128 : (c + 1) * 128],
                    start=(c == 0), stop=False)
            for c in range(HC):
                nc.tensor.matmul(
                    psC_ap, aap_sb[:, c : c + 1],
                    combw_sb[:, (HC + c) * 128 : (HC + c + 1) * 128],
                    start=False, stop=False)
            # bias via K=1 ones matmul, then relu
            nc.tensor.matmul(psC_ap, ones_row[0:1, 0:1], combb_sb[:],
                             start=False, stop=True)
            xrow_sb = spool.tile([1, 128], f32)
            nc.scalar.activation(
                xrow_sb[:], psC_ap, mybir.ActivationFunctionType.Relu)
            # transpose x row -> [128, 1] for use as stationary operand
            psX = pp.tile([128, 48], f32, tag="psA", bufs=2)
            nc.tensor.transpose(psX[:, 0:1], xrow_sb[:], ident[0:1, 0:1])
            xsh_sb = spool.tile([128, 1], mybir.dt.bfloat16)
            nc.vector.tensor_copy(xsh_sb[:], psX[:, 0:1])
            h0own_bf = spool.tile([128, 1], mybir.dt.bfloat16)
            nc.vector.tensor_copy(h0own_bf[:], h0own_sb[:])

            # ---- GRU, contraction-sharded with weights as the moving
            # operand: partial gi = x_chunk.T @ W_ih[:, j].T -> [1, 3072],
            # same for gh; AllReduce(add) of [1, 6144]; transpose-load to
            # [128, 48] chunk layout; elementwise GRU gives FULL h'
            # replicated on every core (no h all-gather).
            # 12 pieces (gi n=0..5, gh n=0..5) in 3 waves of 4 concurrent
            # column-strips; each wave drains with one wide DVE copy and four
            # direct SBUF->DRAM writes into the AllReduce bounce
            ar_in = dram.tile([1, 2 * 3 * NHID], f32)
            ar_out = dram.tile([1, 2 * 3 * NHID], f32, addr_space="Shared")
            for w in range(3):
                psG4 = pp.tile([128, 512], f32, tag="psG", bufs=2)
                for q in range(4):
                    p = 4 * w + q
                    lhs = xsh_sb if p < 6 else h0own_bf
                    n = p % 6
                    wmat = wih_sb if p < 6 else whh_sb
                    nc.tensor.matmul(
                        psG4[32 * q : 32 * q + 1, :],
                        lhs[:], wmat[:, n * 512 : (n + 1) * 512],
                        start=True, stop=True, tile_position=(0, 32 * q))
                gw = spool.tile([128, 512], f32, tag="gw", bufs=2)
                nc.vector.tensor_copy(gw[:], psG4[:])
                for q in range(4):
                    p = 4 * w + q
                    nc.scalar.dma_start(
                        ar_in[0:1, p * 512 : (p + 1) * 512],
                        gw[32 * q : 32 * q + 1, :])
            nc.gpsimd.collective_compute(
                "AllReduce", mybir.AluOpType.add, replica_groups=RG,
                ins=[ar_in[:].opt()], outs=[ar_out[:].opt()],
            )
            # load as [48, 128] (contiguous rows) and transpose to [128, 48]
            g48_sb = spool.tile([48, 128], f32)
            nc.gpsimd.dma_start(g48_sb[:], ar_out[:].rearrange("o (a b) -> (o a) b", b=128))
            psG48 = pp.tile([128, 48], f32, tag="psA", bufs=2)
            nc.tensor.transpose(psG48[:], g48_sb[:], ident[0:48, 0:48])
            gf_sb = spool.tile([128, 48], f32)
            nc.vector.tensor_copy(gf_sb[:], psG48[:])
            # cols: 0:8 gi_r, 8:16 gi_z, 16:24 gi_n, 24:32 gh_r, 32:40 gh_z, 40:48 gh_n

            # elementwise GRU on [128, HC] chunk-layout tensors
            brz_sb = spool.tile([128, 2 * HC], f32)
            nc.vector.tensor_add(brz_sb[:], bih_sb[:, 0 : 2 * HC], bhh_sb[:, 0 : 2 * HC])
            rz0_sb = spool.tile([128, 2 * HC], f32)
            nc.vector.tensor_add(rz0_sb[:], gf_sb[:, 0 : 2 * HC], gf_sb[:, 3 * HC : 5 * HC])
            rzin_sb = spool.tile([128, 2 * HC], f32)
            nc.vector.tensor_add(rzin_sb[:], rz0_sb[:], brz_sb[:])
            rz_sb = spool.tile([128, 2 * HC], f32)
            nc.scalar.activation(
                rz_sb[:], rzin_sb[:], mybir.ActivationFunctionType.Sigmoid)
            hnb_sb = spool.tile([128, HC], f32)
            nc.vector.tensor_add(
                hnb_sb[:], gf_sb[:, 5 * HC : 6 * HC], bhh_sb[:, 2 * HC : 3 * HC])
            rhn_sb = spool.tile([128, HC], f32)
            nc.vector.tensor_mul(rhn_sb[:], rz_sb[:, 0:HC], hnb_sb[:])
            t1_sb = spool.tile([128, HC], f32)
            nc.vector.tensor_add(t1_sb[:], gf_sb[:, 2 * HC : 3 * HC], rhn_sb[:])
            t2_sb = spool.tile([128, HC], f32)
            nc.vector.tensor_add(t2_sb[:], t1_sb[:], bih_sb[:, 2 * HC : 3 * HC])
            n_sb = spool.tile([128, HC], f32)
            nc.scalar.activation(
                n_sb[:], t2_sb[:], mybir.ActivationFunctionType.Tanh)
            d_sb = spool.tile([128, HC], f32)
            nc.vector.tensor_sub(d_sb[:], h0c_sb[:], n_sb[:])
            zd_sb = spool.tile([128, HC], f32)
            nc.vector.tensor_mul(zd_sb[:], rz_sb[:, HC : 2 * HC], d_sb[:])
            hn_sb = spool.tile([128, HC], f32)
            nc.vector.tensor_add(hn_sb[:], n_sb[:], zd_sb[:])
            nc.scalar.dma_start(h_out[:], hn_sb[:])
            # preload Exp/Ln activation tables off the critical tail
            tbl_scr = spool.tile([1, 1], f32)
            nc.scalar.activation(tbl_scr[:], hn_sb[0:1, 0:1],
                                 mybir.ActivationFunctionType.Exp)
            tbl_scr2 = spool.tile([1, 1], f32)
            nc.scalar.activation(tbl_scr2[:], ones_row[0:1, 0:1],
                                 mybir.ActivationFunctionType.Ln)
            h_mm = cpool.tile([128, HC], wdt)
            nc.vector.tensor_copy(h_mm[:], hn_sb[:])

            # ---- out projection (streamed, vocab shard VS=6400, 16 tiles)
            # bias folded into the matmul accumulation via a K=1 ones matmul;
            # per-tile online softmax stats on partition 0 overlap the stream.
            logits_sb = cpool.tile([16, TN], f32)
            for quad in range(NT // 4):
                w_a = wpool.tile([128, WG * HC * TN], wdt, tag="wtile")
                nc.sync.dma_start(w_a[:], wout_in[2 * quad])
                w_b = wpool.tile([128, WG * HC * TN], wdt, tag="wtile")
                nc.sync.dma_start(w_b[:], wout_in[2 * quad + 1])
                # 4-way PE column-tiling: 4 tiles run in disjoint 32-column
                # strips of the array concurrently
                psT4 = pp.tile([128, TN], f32, tag="psT", bufs=2)
                for k in range(HC):
                    for q in range(4):
                        w_t = w_a if q < 2 else w_b
                        nc.tensor.matmul(
                            psT4[32 * q : 32 * q + 1, :],
                            h_mm[:, k : k + 1],
                            w_t[:, ((q % 2) * HC + k) * TN : ((q % 2) * HC + k + 1) * TN],
                            start=(k == 0),
                            stop=(k == HC - 1),
                            tile_position=(0, 32 * q),
                        )
                # one full-width DVE copy drains all 4 rows (lanes in parallel)
                lwide = spool.tile([128, TN], f32, tag="lwide", bufs=2)
                nc.vector.tensor_copy(lwide[:], psT4[:])
                for q in range(4):
                    t = quad * 4 + q
                    # sync ring is idle once the W chunks are issued
                    nc.sync.dma_start(
                        logits_sb[t : t + 1, :], lwide[32 * q : 32 * q + 1, :])

            # ---- combine the 16 per-tile stats (partition 0) ----
            logits2_sb = cpool.tile([16, TN], f32)
            nc.vector.tensor_add(logits2_sb[:], logits_sb[:], bout_sb[:])
            # per-row stats straight into the AllGather bounce: col0 = -max,
            # col1 = sum(exp(l - max)); the global merge happens over all
            # 128 (core, row) pairs after the gather — no local merge chain
            stats16 = spool.tile([16, 2], f32)
            nc.vector.reduce_max(
                stats16[:, 0:1], logits2_sb[:], axis=mybir.AxisListType.X,
                negate=True)
            e16 = spool.tile([16, TN], f32, tag="row16", bufs=1)
            nc.scalar.activation(
                e16[:], logits2_sb[:], mybir.ActivationFunctionType.Exp,
                bias=stats16[:, 0:1], accum_out=stats16[:, 1:2])

            # ---- AllGather per-row stats ----
            st_in = dram.tile([16, 2], f32)
            st_out = dram.tile([16 * NCORES, 2], f32, addr_space="Shared")
            nc.gpsimd.dma_start(st_in[:], stats16[:])
            nc.gpsimd.collective_compute(
                "AllGather", mybir.AluOpType.bypass, replica_groups=RG,
                ins=[st_in[:].opt()], outs=[st_out[:].opt()],
            )
            s16_sb = spool.tile([1, 2 * 16 * NCORES], f32)
            nc.gpsimd.dma_start(
                s16_sb[:], st_out[:].rearrange("(o a) b -> o (a b)", o=1))

            # global max M, then C = M + ln(sum_c Z_c exp(m_c - M))
            s16v = s16_sb[:].rearrange("p (a b) -> p a b", b=2)
            ngM = spool.tile([1, 1], f32)   # = -M_global (min of negated maxes)
            nc.vector.tensor_reduce(
                ngM[:], s16v[:, :, 0:1], op=mybir.AluOpType.min,
                axis=mybir.AxisListType.XY)
            mp = spool.tile([1, 16 * NCORES], f32)
            nc.vector.tensor_scalar_mul(mp[:], s16v[:, :, 0:1], -1.0)
            e8 = spool.tile([1, 16 * NCORES], f32)
            nc.scalar.activation(
                e8[:], mp[:], mybir.ActivationFunctionType.Exp,
                bias=ngM[0:1, 0:1])
            s8p = spool.tile([1, 16 * NCORES], f32)
            nc.vector.tensor_mul(s8p[:], e8[:], s16v[:, :, 1:2])
            Zg = spool.tile([1, 1], f32)
            nc.vector.reduce_sum(Zg[:], s8p[:], axis=mybir.AxisListType.X)
            lnZ = spool.tile([1, 1], f32)
            nc.scalar.activation(lnZ[:], Zg[:], mybir.ActivationFunctionType.Ln)
            nC = spool.tile([1, 1], f32)
            nc.vector.tensor_sub(nC[:], ngM[:], lnZ[:])
            psB3 = pp.tile([16, 1], f32, tag="psS", bufs=1)
            nc.tensor.matmul(psB3[:], ones_row[0:1, 0:16], nC[:], start=True, stop=True)
            nC16_sb = spool.tile([16, 1], f32)
            nc.vector.tensor_copy(nC16_sb[:], psB3[:])
            logp_sb = spool.tile([16, TN], f32, tag="row16", bufs=1)
            nc.scalar.activation(
                logp_sb[:], logits2_sb[:], mybir.ActivationFunctionType.Identity,
                bias=nC16_sb[:, 0:1])
            nc.sync.dma_start(logp_out[:], logp_sb[:])

    nc.compile()
    return nc


def _prep_inputs(inp, hidden, encoder_outputs, emb_W, attn_W, attn_b,
                 comb_W, comb_b, W_ih, W_hh, b_ih, b_hh, out_W, out_b):
    """Shard/layout the full inputs into 8 per-core input maps."""
    f = np.float32
    idx = int(np.asarray(inp).ravel()[0])
    emb_row = np.asarray(emb_W[idx], dtype=f)                 # [1024]
    h0 = np.asarray(hidden, dtype=f).ravel()                  # [1024]
    enc = np.ascontiguousarray(np.asarray(encoder_outputs, dtype=f))  # [24,1024]

    def chunked_vec(v):
        # [1024] -> [128, 8] with [p, c] = v[c*128+p]
        return np.ascontiguousarray(v.reshape(HC, 128).T)

    emb_c = chunked_vec(emb_row)
    h0_c = chunked_vec(h0)

    # attn_W [24, 2048] -> [128, 16*24]
    aT = np.asarray(attn_W, dtype=f).T.reshape(16, 128, MAX_LEN)
    attnw = np.ascontiguousarray(aT.transpose(1, 0, 2).reshape(128, 16 * MAX_LEN))
    attnb = np.ascontiguousarray(np.asarray(attn_b, dtype=f).reshape(1, MAX_LEN))

    comb_W = np.asarray(comb_W, dtype=f)
    comb_b_a = np.asarray(comb_b, dtype=f)
    W_ih_a = np.asarray(W_ih, dtype=f)
    W_hh_a = np.asarray(W_hh, dtype=f)
    b_ih_a = np.asarray(b_ih, dtype=f)
    b_hh_a = np.asarray(b_hh, dtype=f)
    out_W_a = np.asarray(out_W, dtype=f)
    out_b_a = np.asarray(out_b, dtype=f)

    if WOUT_BF16:
        import ml_dtypes

        wout_dt = ml_dtypes.bfloat16
    else:
        wout_dt = f

    in_maps = []
    for j in range(NCORES):
        sl = slice(j * 128, (j + 1) * 128)
        # comb shard [128, 2048] -> [128(p), 16*128]
        cw = comb_W[sl]                                    # [128, 2048]
        cwT = cw.T.reshape(16, 128, 128)                   # [c, p, m]
        combw = np.ascontiguousarray(cwT.transpose(1, 0, 2).reshape(128, 16 * 128))
        combb = np.ascontiguousarray(comb_b_a[sl].reshape(1, 128))

        def gate_pack(W):
            # moving-operand layout: rhs[p, q] = W[q, j*128+p]
            import ml_dtypes
            return np.ascontiguousarray(
                W[:, j * 128 : (j + 1) * 128].T.astype(ml_dtypes.bfloat16))

        wih = gate_pack(W_ih_a)
        whh = gate_pack(W_hh_a)

        def bias_pack(b):
            # [128, 3*HC]: col g*HC+c holds b[g*1024 + c*128 + p]
            return np.ascontiguousarray(
                b.reshape(3, HC, 128).transpose(2, 0, 1).reshape(128, 3 * HC))

        bih = bias_pack(b_ih_a)
        bhh = bias_pack(b_hh_a)

        # out_W vocab shard [VS, 1024] (zero-padded), bias shard with PAD_BIAS
        lo, hi = j * VS, min((j + 1) * VS, NOUT)
        nreal = max(0, hi - lo)
        wsh = np.zeros((VS, NHID), dtype=f)
        bsh = np.full((VS,), PAD_BIAS, dtype=f)
        if nreal > 0:
            wsh[:nreal] = out_W_a[lo:hi]
            bsh[:nreal] = out_b_a[lo:hi]
        WT = wsh.T                                         # [1024, 6400]
        # [NT, 128, HC*TN]: [t, p, k*TN+n] = WT[k*128+p, t*TN+n], grouped by WG
        warr = (
            WT.reshape(HC, 128, NT, TN).transpose(2, 1, 0, 3)
            .reshape(NT // WG, WG, 128, HC * TN).transpose(0, 2, 1, 3)
            .reshape(NT // WG, 128, WG * HC * TN)
        )
        warr = np.ascontiguousarray(warr.astype(wout_dt))
        barr = np.ascontiguousarray(bsh.reshape(16, TN))

        pack = np.zeros((128, 2649), dtype=f)
        pack[:, 0:HC] = emb_c
        pack[:, HC : 2 * HC] = h0_c
        pack[:, 2 * HC : 2 * HC + 1] = h0[sl].reshape(128, 1)
        pack[:, 17:41] = bih
        pack[:, 41:65] = bhh
        pack[:, 65:449] = attnw
        pack[:, 449:2497] = combw
        pack[0, 2497 : 2497 + MAX_LEN] = attnb.ravel()
        pack[0, 2497 + MAX_LEN : 2649] = combb.ravel()
        in_maps.append({
            "pack_in": np.ascontiguousarray(pack),
            "enc_in": enc,
            "wih_in": wih, "whh_in": whh,
            "wout_in": warr, "bout_in": barr,
        })
    return in_maps


def run(trace=False, **inputs):
    from concourse.bass_utils import run_bass_kernel_spmd

    if "nc" not in _CACHE:
        _CACHE["nc"] = _build()
    nc = _CACHE["nc"]

    inputs.pop("encoder_output", None)  # unused by the reference computation
    in_maps = _prep_inputs(**inputs)
    res = run_bass_kernel_spmd(
        nc, in_maps, core_ids=list(range(NCORES)), trace=trace
    )

    logp = np.concatenate(
        [res.results[j]["logp_out"].reshape(-1) for j in range(NCORES)]
    )[:NOUT].reshape(1, NOUT).astype(np.float32)
    # h_out is [128, HC] chunk layout, full h' replicated on every core
    h = res.results[0]["h_out"].T.reshape(1, 1, NHID).astype(np.float32)
    attn = res.results[0]["attn_out"].reshape(1, MAX_LEN).astype(np.float32)
    return (logp, h, attn), res


def kernel(**inputs):
    out, _ = run(trace=bool(os.environ.get("KERNEL_TRACE")), **inputs)
    return out
